# revision 6
# baseline (speedup 1.0000x reference)
"""Multi-head attention (BN-folded QKV + rel-pos bias + GELU + out-proj) on 8 TRN2 cores.

Data-parallel over batch (b=8 -> 1 batch element per core, no collectives).

All BatchNorms are eval-mode affine transforms folded into the projection
weights/biases on the host.  The additive Toeplitz position bias is folded in
multiplicatively after exp:  exp(dots + E) = exp(dots) * exp(E), with exp(E)
shipped as per-partition pre-shifted window tiles so each [j, i] tile of the
attention matrix multiplies a plain strided SBUF view.

Layout choice: dots are computed transposed (dotsT[j, i]) so q/k feed the
TensorEngine exactly as the projections produce them, v is produced already
transposed ([j, dv]) by swapping matmul operands, and a ones-column in v
makes the softmax denominators fall out of the attn@v matmul for free.

Schedule: ACT (exp) is the pace-setting engine (64 x [128,1024] exps = 73us
floor), so everything else is arranged to never stall it: i-halves are
processed as outer phases so each (pair, ihalf) accumulator is only 2 PSUM
banks and the 2-deep acc ring hands off across phases without a stall (the
old per-pair 4-bank accs exhausted PSUM and idled the PE at pair boundaries
long enough for HAM to re-throttle the clock to 1.2GHz for ~7us at a time);
projections are interleaved into the attention stream through the dots ring
instead of running up-front in the PE's in-order queue; the window multiply
runs at 2-unit granularity alternating between DVE and the otherwise-idle
GpSimd engine; PSUM evacuations ride DVE, keeping ACT exp-only mid-stream.
"""

import numpy as np
import ml_dtypes

HEADS, DK, DV = 8, 32, 64
DIM, N, DIM_OUT = 256, 1024, 256
IDK, IDV = DK * HEADS, DV * HEADS  # 256, 512
SCALE = DK ** -0.5
EPS = 1e-5
B = 8
WIN = 1920  # window tile width: covers i - 128*jc in [-896, 1023]

BF16 = ml_dtypes.bfloat16


def _prep_host(x, Wq, Wk, Wv, Wo, bo, pos_emb,
               q_gamma, q_beta, q_mean, q_var,
               k_gamma, k_beta, k_mean, k_var,
               v_gamma, v_beta, v_mean, v_var,
               o_gamma, o_beta, o_mean, o_var):
    f32 = np.float32
    inv_q = (q_gamma / np.sqrt(q_var + EPS)).astype(f32)
    inv_k = (k_gamma / np.sqrt(k_var + EPS)).astype(f32)
    inv_v = (v_gamma / np.sqrt(v_var + EPS)).astype(f32)
    inv_o = (o_gamma / np.sqrt(o_var + EPS)).astype(f32)

    # q also absorbs the attention scale
    Wq_eff = (Wq * inv_q[:, None]) * SCALE
    bq = ((q_beta - q_mean * inv_q) * SCALE).astype(f32)
    Wk_eff = Wk * inv_k[:, None]
    bk = (k_beta - k_mean * inv_k).astype(f32)
    Wv_eff = Wv * inv_v[:, None]
    bv = (v_beta - v_mean * inv_v).astype(f32)
    Wo_eff = Wo * inv_o[:, None]
    bo_eff = ((bo - o_mean) * inv_o + o_beta).astype(f32)

    # lhsT layouts, pre-chunked to the exact SBUF tile shapes
    def chunk_T(w, kchunks):  # [O, C] -> [128, kchunks, O]  (WT[c, o] tiled)
        wT = np.ascontiguousarray(w.T.astype(f32))  # [C, O]
        c, o = wT.shape
        assert c == kchunks * 128
        return np.ascontiguousarray(
            wT.reshape(kchunks, 128, o).transpose(1, 0, 2)).astype(BF16)

    wqs = chunk_T(Wq_eff, 2)            # [128, 2, 256]
    wks = chunk_T(Wk_eff, 2)            # [128, 2, 256]
    wvs = chunk_T(Wv_eff, 2)            # [128, 2, 512]
    wos = chunk_T(Wo_eff, 4)            # [128, 4, 256]

    def chunk_bias(b, chunks):  # [C] -> [128, chunks]
        return np.ascontiguousarray(
            b.reshape(chunks, 128).T).astype(f32)

    bos = chunk_bias(bo_eff, 2)         # [128, 2]
    # q/k biases enter the projection as a K=1 ones-row matmul
    bqrow = bq.reshape(1, IDK).astype(BF16)  # [1, 256]
    bkrow = bk.reshape(1, IDK).astype(BF16)  # [1, 256]
    bvrow = bv.reshape(1, IDV).astype(BF16)  # [1, 512]  (h-major: c = h*64+dv)

    # exp-window tiles: indexed so that for j = 128*jc + p, attn[p, i] needs
    # expE[1023 + i - j] = win[p, h, (896 - 128*jc) + i]
    E = (np.asarray(pos_emb, dtype=np.float64) / SCALE)  # [N, HEADS]
    d = np.abs(np.arange(2047) - 1023)
    expE = np.exp(E[d, :])  # [2047, HEADS] float64
    idx = np.arange(WIN)[None, :] - np.arange(128)[:, None] + 127  # [128, WIN]
    win = expE[idx, :].transpose(0, 2, 1)  # [128, HEADS, WIN]
    win = np.ascontiguousarray(win).astype(BF16)

    shared = dict(wqs=wqs, wks=wks, wvs=wvs, wos=wos,
                  bqrow=bqrow, bkrow=bkrow, bos=bos, bvrow=bvrow, win=win)
    return shared


def _x_shard(x, i):
    # device consumes x as bf16 [128, 2, n] (channel chunks c = a*128 + p)
    xi = np.asarray(x[i], dtype=np.float32).reshape(2, 128, N).transpose(1, 0, 2)
    return np.ascontiguousarray(xi).astype(BF16)


def _build_nc():
    import concourse.bass as bass
    import concourse.mybir as mybir
    import concourse.tile as tile
    from concourse import bacc

    f32 = mybir.dt.float32
    bf16 = mybir.dt.bfloat16

    nc = bacc.Bacc(None, target_bir_lowering=False)

    x_ext = nc.declare_dram_parameter("x", [128, 2, N], bf16, isOutput=False)
    wqs_ext = nc.declare_dram_parameter("wqs", [128, 2, IDK], bf16, isOutput=False)
    wks_ext = nc.declare_dram_parameter("wks", [128, 2, IDK], bf16, isOutput=False)
    wvs_ext = nc.declare_dram_parameter("wvs", [128, 2, IDV], bf16, isOutput=False)
    wos_ext = nc.declare_dram_parameter("wos", [128, 4, DIM_OUT], bf16, isOutput=False)
    bqrow_ext = nc.declare_dram_parameter("bqrow", [1, IDK], bf16, isOutput=False)
    bkrow_ext = nc.declare_dram_parameter("bkrow", [1, IDK], bf16, isOutput=False)
    bos_ext = nc.declare_dram_parameter("bos", [128, 2], f32, isOutput=False)
    bvrow_ext = nc.declare_dram_parameter("bvrow", [1, IDV], bf16, isOutput=False)
    win_ext = nc.declare_dram_parameter("win", [128, HEADS, WIN], bf16, isOutput=False)
    out_ext = nc.declare_dram_parameter("out", [DIM_OUT, N], f32, isOutput=True)

    Exp = mybir.ActivationFunctionType.Exp
    Gelu = mybir.ActivationFunctionType.Gelu

    with tile.TileContext(nc) as tc:
        with (
            tc.tile_pool(name="consts", bufs=1) as consts,
            tc.tile_pool(name="scratch", bufs=2) as scratch,
            tc.tile_pool(name="attnp", bufs=6) as attnp,
            tc.tile_pool(name="normp", bufs=6) as normp,
            tc.tile_pool(name="psum", bufs=2, space="PSUM") as psum,
        ):
            # ---- load constants, in consumption order on one queue.  The
            # exp-window table (3.9MB, half the input bytes) is split into
            # per-pair chunks issued behind everything a chunk's first reader
            # doesn't need, so the early pipeline is never DMA-starved. ----
            xb = consts.tile([128, 2, N], bf16)
            nc.sync.dma_start(out=xb, in_=x_ext[:])
            wq = consts.tile([128, 2, IDK], bf16)
            nc.sync.dma_start(out=wq, in_=wqs_ext[:])
            wk = consts.tile([128, 2, IDK], bf16)
            nc.sync.dma_start(out=wk, in_=wks_ext[:])
            bqr = consts.tile([1, IDK], bf16)
            nc.sync.dma_start(out=bqr, in_=bqrow_ext[:])
            bkr = consts.tile([1, IDK], bf16)
            nc.sync.dma_start(out=bkr, in_=bkrow_ext[:])
            wv = consts.tile([128, 2, IDV], bf16)
            nc.sync.dma_start(out=wv, in_=wvs_ext[:])
            bvr = consts.tile([1, IDV], bf16)
            nc.sync.dma_start(out=bvr, in_=bvrow_ext[:])
            win = consts.tile([128, HEADS, WIN], bf16)
            win_ranges = [(768, 1920), (512, 768), (256, 512), (0, 256)]
            nc.sync.dma_start(out=win[:, 0:2, 768:1920],
                              in_=win_ext[:, 0:2, 768:1920])
            wo = consts.tile([128, 4, DIM_OUT], bf16)
            nc.sync.dma_start(out=wo, in_=wos_ext[:])
            bos = consts.tile([128, 2], f32)
            nc.sync.dma_start(out=bos, in_=bos_ext[:])
            for p in range(4):
                for (u0, u1) in win_ranges[1:] if p == 0 else win_ranges:
                    nc.sync.dma_start(out=win[:, 2 * p:2 * p + 2, u0:u1],
                                      in_=win_ext[:, 2 * p:2 * p + 2, u0:u1])
            ones1 = consts.tile([1, 512], bf16)
            nc.vector.memset(ones1, 1.0)
            # dummy exp: walrus inserts the exp table load before ACT's first
            # Exp -- placing one here pulls the ~2.7us load into the DMA wait
            # instead of the first attention pair's critical path
            warm = scratch.tile([1, 8], f32, tag="warm")
            nc.vector.memset(warm, 1.0)
            nc.scalar.activation(warm, warm, Exp)

            # ---- persistent intermediates ----
            q_sb = consts.tile([128, 2, N], bf16)   # [ (h,d) chunks, i ]
            k_sb = consts.tile([128, 2, N], bf16)   # [ (h,d) chunks, j ]
            # v columns 64:128 per head, columns 0:32 all-ones (sums come out
            # 32-replicated at base partition 0 -- safe for the custom-DVE
            # reciprocal), columns 32:64 zero (dark PE cells), out_u on
            # partitions 1:65.  M=65 keeps half the PE array dark vs M=128 --
            # same cycles, less power for the activity throttler to punish.
            v_aug = consts.tile([128, 8, HEADS, 128], bf16)  # [j-part, jc, h, one|0|dv]
            g_sb = consts.tile([128, 4, N], bf16)   # gelu input/output [(h,dv) chunks, i]

            nc.gpsimd.memset(v_aug[:, :, :, 0:32], 1.0)
            nc.gpsimd.memset(v_aug[:, :, :, 32:DV], 0.0)

            # ---- projections.  Each is one dots-ring pseudo-unit: 3 PE
            # matmuls into [:, 0:512] of a ring slot, evacuated by DVE so the
            # ACT queue stays exp-only mid-stream. ----
            def emit_qkproj(mc, w_t, b_r, dst, ic):
                ps_t = psum.tile([128, N], f32, tag="dots", bufs=2,
                                 name=f"qk_{mc}_{ic}")
                ps = ps_t[:, 0:512]
                for kc in range(2):
                    nc.tensor.matmul(
                        ps,
                        lhsT=w_t[:, kc, mc * 128:(mc + 1) * 128],
                        rhs=xb[:, kc, ic * 512:(ic + 1) * 512],
                        start=(kc == 0), stop=False)
                nc.tensor.matmul(
                    ps, lhsT=b_r[:, mc * 128:(mc + 1) * 128],
                    rhs=ones1[:, 0:512], start=False, stop=True)
                nc.vector.tensor_copy(dst[:, mc, ic * 512:(ic + 1) * 512], ps)

            def emit_vproj(jc):
                ps_t = psum.tile([128, N], f32, tag="dots", bufs=2,
                                 name=f"vps_{jc}")
                ps = ps_t[:, 0:512]
                for kc in range(2):
                    nc.tensor.matmul(
                        ps,
                        lhsT=xb[:, kc, jc * 128:(jc + 1) * 128],
                        rhs=wv[:, kc, :],
                        start=(kc == 0), stop=False)
                # + bias via ones-row x bvrow (K=1)
                nc.tensor.matmul(ps, lhsT=ones1[:, 0:128], rhs=bvr,
                                 start=False, stop=True)
                nc.vector.tensor_copy(
                    v_aug[:, jc, :, DV:128],
                    ps.rearrange("p (h d) -> p h d", h=HEADS))

            # PE warm-up: dense K=128 matmuls on garbage SBUF bridge the
            # input-DMA wait so HAM sees activity before real work lands
            wps = psum.tile([128, N], f32, tag="dots", bufs=2, name="warmps")
            for _ in range(4):
                nc.tensor.matmul(wps[:, 0:512], lhsT=q_sb[:, 0, 0:128],
                                 rhs=q_sb[:, 0, 0:512], start=True, stop=True)

            # minimal pre-attention batch: just what dots(p0, ic0, jc0..3)
            # needs; everything else interleaves into the attention stream
            emit_qkproj(0, wq, bqr, q_sb, 0)
            emit_qkproj(0, wk, bkr, k_sb, 0)
            emit_vproj(0)
            emit_vproj(1)

            # remaining projection units, interleaved into the attention
            # stream: (emit_fn, deadline = global unit index whose body needs
            # the result emitted first), sorted by deadline
            proj_queue = [
                (lambda: emit_qkproj(0, wk, bkr, k_sb, 1), 4),   # dots jc4
                (lambda: emit_vproj(2), 5),
                (lambda: emit_vproj(3), 5),
                (lambda: emit_vproj(4), 7),
                (lambda: emit_vproj(5), 7),
                (lambda: emit_qkproj(0, wq, bqr, q_sb, 1), 8),   # p0 ic1
                (lambda: emit_vproj(6), 9),
                (lambda: emit_vproj(7), 9),
                (lambda: emit_qkproj(1, wq, bqr, q_sb, 0), 31),  # p2+
                (lambda: emit_qkproj(1, wk, bkr, k_sb, 0), 31),
                (lambda: emit_qkproj(1, wk, bkr, k_sb, 1), 33),
                (lambda: emit_qkproj(1, wq, bqr, q_sb, 1), 39),
            ]

            # ---- attention: phases (pair, ihalf); units are jc 0..7 inside a
            # phase.  Two heads' K=32 dots matmuls share the PE array via
            # distinct 32-row groups (concurrent).  attn tiles pair two
            # consecutive jc so the window multiply runs at [128, 2048]
            # granularity; pairs alternate between DVE and GpSimd. ----
            import concourse.bass as bass_mod

            pending_norms = []

            def emit_norm_chain(p, ic, hl, ops):
                # g[dv, i-half] = out_u[dv, i-half] / sums[i-half]
                h = 2 * p + hl
                bc = normp.tile([DV, 512], f32, tag="bc", name=f"bc_{h}_{ic}")
                nc.vector.reciprocal_approx_fast(bc[0:32, :], ops[0:32, hl, :])
                nc.sync.dma_start(out=bc[32:DV, :], in_=bc[0:32, :])
                nc.vector.tensor_mul(
                    g_sb[hl * DV:hl * DV + DV, p, ic * 512:(ic + 1) * 512],
                    ops[DV:128, hl, :], bc)

            attnv_q = []

            def emit_attnv(p, ic, jc, acc, at, sl):
                for hl in range(2):
                    nc.tensor.matmul(
                        acc[:, hl, :],
                        lhsT=v_aug[:, jc, 2 * p + hl, :],
                        rhs=at[:, sl, hl, :],
                        start=(jc == 0), stop=(jc == 7))

            phases = [(p, ic) for p in range(4) for ic in range(2)]
            units = [(p, ic, jc) for (p, ic) in phases for jc in range(8)]
            acc_of = {}
            attn2 = None
            last_attn = None
            u = 0
            for p, ic, jc in units:
                h0, h1 = 2 * p, 2 * p + 1
                koff0, kch0 = (h0 % 4) * 32, h0 // 4
                koff1, kch1 = (h1 % 4) * 32, h1 // 4
                emitted = False
                while proj_queue and proj_queue[0][1] <= u:
                    proj_queue.pop(0)[0]()
                    emitted = True
                if proj_queue and not emitted and u % 3 == 2:
                    proj_queue.pop(0)[0]()
                if jc == 0:
                    acc_of[(p, ic)] = psum.tile(
                        [128, 2, 512], f32, tag="acc", bufs=2,
                        name=f"acc_{p}_{ic}")
                acc = acc_of[(p, ic)]
                if jc % 2 == 0:
                    attn2 = attnp.tile([128, 2, 2, 512], bf16, tag="attn",
                                       name=f"attn_{p}_{ic}_{jc}")
                # slot assignment: odd jc -> slot 0, even jc -> slot 1, so the
                # window view's pair dim can step +128 (odd jc has the
                # smaller window offset; negative AP steps avoided)
                sl = 1 - (jc % 2)
                dps = psum.tile([128, N], f32, tag="dots", bufs=2,
                                name=f"dots_{p}_{ic}_{jc}")
                nc.tensor.matmul(
                    dps[:, 0:512],
                    lhsT=k_sb[koff0:koff0 + 32, kch0, jc * 128:(jc + 1) * 128],
                    rhs=q_sb[koff0:koff0 + 32, kch0, ic * 512:(ic + 1) * 512],
                    start=True, stop=True, tile_position=(koff0, 0))
                nc.tensor.matmul(
                    dps[:, 512:1024],
                    lhsT=k_sb[koff1:koff1 + 32, kch1, jc * 128:(jc + 1) * 128],
                    rhs=q_sb[koff1:koff1 + 32, kch1, ic * 512:(ic + 1) * 512],
                    start=True, stop=True, tile_position=(koff1, 0))
                nc.scalar.activation(attn2[:, sl], dps.rearrange(
                    "p (h i) -> p h i", h=2), Exp)
                if jc % 2 == 1:
                    # one window multiply per unit-pair; attn@v consumers lag
                    # a pair behind, so waiting on both exps costs no latency
                    off = 896 - 128 * jc + 512 * ic
                    wv_view = win[:, h0, off:off + 512]
                    wv_quad = bass_mod.AP(
                        tensor=wv_view.tensor, offset=wv_view.offset,
                        ap=[list(wv_view.ap[0]), [128, 2], [WIN, 2], [1, 512]])
                    # wv_quad dim 1: index 0 = this (odd) jc's offset,
                    # index 1 = +128 = the even jc before it -- matching the
                    # slot convention above
                    eng = nc.vector if ((u // 2) % 2 == 0) else nc.gpsimd
                    eng.tensor_mul(attn2, attn2, wv_quad)
                    last_attn = attn2
                    attnv_q.append((p, ic, jc - 1, acc, attn2, 1))
                    attnv_q.append((p, ic, jc, acc, attn2, 0))
                    while len(attnv_q) > 2:
                        emit_attnv(*attnv_q.pop(0))
                if pending_norms and jc >= 2:
                    emit_norm_chain(*pending_norms.pop(0))
                if jc == 7:
                    pending_norms += [(p, ic, 0, acc), (p, ic, 1, acc)]
                u += 1
            while attnv_q:
                emit_attnv(*attnv_q.pop(0))
            for fn, _ in proj_queue:
                fn()

            # ---- tail: the last phase's norms (DVE) run while ACT loads the
            # gelu table (warm-gelu gated on the last attn tile so the load
            # can't hoist into the exp stream), then gelu + out-projection
            # pipeline through the freed acc ring. ----
            out_r = out_ext[:].rearrange("(a p) n -> p a n", p=128)
            for args in pending_norms:
                emit_norm_chain(*args)
            pending_norms = []
            # zero-valued gelu bias derived from the LAST attn tile: a pure
            # scheduling gate keeping the gelus (and their table load) after
            # the final exp so the exp/gelu table sets swap exactly once.
            zg = scratch.tile([128, 1], f32, tag="zg")
            nc.vector.tensor_scalar_mul(zg, last_attn[:, 0, 0, 0:1], 0.0)
            fin = {mc: psum.tile([128, N], f32, tag="acc", bufs=2,
                                 name=f"fin_{mc}") for mc in range(2)}
            for cp in range(2):  # gelu chunk-pairs: heads 0-3, then 4-7
                gsl = g_sb[:, 2 * cp:2 * cp + 2, :]
                nc.scalar.activation(gsl, gsl, Gelu, bias=zg)
                for mc in range(2):
                    for ic in range(2):
                        for kc in (2 * cp, 2 * cp + 1):
                            nc.tensor.matmul(
                                fin[mc][:, ic * 512:(ic + 1) * 512],
                                lhsT=wo[:, kc, mc * 128:(mc + 1) * 128],
                                rhs=g_sb[:, kc, ic * 512:(ic + 1) * 512],
                                start=(kc == 0), stop=(kc == 3))
            for mc in range(2):
                o_sb = scratch.tile([128, N], f32, tag="osb",
                                    bufs=2, name=f"osb_{mc}")
                nc.vector.tensor_scalar_add(o_sb, fin[mc], bos[:, mc:mc + 1])
                nc.sync.dma_start(out=out_r[:, mc, :], in_=o_sb)

    nc.finalize()
    return nc


_NC_CACHE = None


def kernel(**inputs) -> np.ndarray:
    global _NC_CACHE
    from concourse.bass_utils import run_bass_kernel_spmd

    x = np.asarray(inputs["x"], dtype=np.float32)
    shared = _prep_host(**inputs)

    if _NC_CACHE is None:
        _NC_CACHE = _build_nc()
    nc = _NC_CACHE

    in_maps = [dict(x=_x_shard(x, i), **shared) for i in range(B)]
    res = run_bass_kernel_spmd(nc, in_maps, core_ids=list(range(B)))
    out = np.stack([res.results[i]["out"] for i in range(B)], axis=0)
    return out.astype(np.float32)


# revision 8
# speedup vs baseline: 1.0545x; 1.0545x over previous
"""Multi-head attention (BN-folded QKV + rel-pos bias + GELU + out-proj) on 8 TRN2 cores.

Data-parallel over batch (b=8 -> 1 batch element per core, no collectives).

All BatchNorms are eval-mode affine transforms folded into the projection
weights/biases on the host.  The additive Toeplitz position bias is folded in
multiplicatively after exp:  exp(dots + E) = exp(dots) * exp(E), with exp(E)
shipped as per-partition pre-shifted window tiles so each [j, i] tile of the
attention matrix multiplies a plain strided SBUF view.

Layout choice: dots are computed transposed (dotsT[j, i]) so q/k feed the
TensorEngine exactly as the projections produce them, v is produced already
transposed ([j, dv]) by swapping matmul operands, and a ones-column in v
makes the softmax denominators fall out of the attn@v matmul for free.

Schedule: ACT (exp) is the pace-setting engine (64 x [128,1024] exps = 73us
floor), so everything else is arranged to never stall it: i-halves are
processed as outer phases so each (pair, ihalf) accumulator is only 2 PSUM
banks and the 2-deep acc ring hands off across phases without a stall (the
old per-pair 4-bank accs exhausted PSUM and idled the PE at pair boundaries
long enough for HAM to re-throttle the clock to 1.2GHz for ~7us at a time);
projections are interleaved into the attention stream through the dots ring
instead of running up-front in the PE's in-order queue; the window multiply
runs at 2-unit granularity alternating between DVE and the otherwise-idle
GpSimd engine; PSUM evacuations ride DVE, keeping ACT exp-only mid-stream.
"""

import numpy as np
import ml_dtypes

HEADS, DK, DV = 8, 32, 64
DIM, N, DIM_OUT = 256, 1024, 256
IDK, IDV = DK * HEADS, DV * HEADS  # 256, 512
SCALE = DK ** -0.5
EPS = 1e-5
B = 8
WIN = 1920  # window tile width: covers i - 128*jc in [-896, 1023]

BF16 = ml_dtypes.bfloat16


def _prep_host(x, Wq, Wk, Wv, Wo, bo, pos_emb,
               q_gamma, q_beta, q_mean, q_var,
               k_gamma, k_beta, k_mean, k_var,
               v_gamma, v_beta, v_mean, v_var,
               o_gamma, o_beta, o_mean, o_var):
    f32 = np.float32
    inv_q = (q_gamma / np.sqrt(q_var + EPS)).astype(f32)
    inv_k = (k_gamma / np.sqrt(k_var + EPS)).astype(f32)
    inv_v = (v_gamma / np.sqrt(v_var + EPS)).astype(f32)
    inv_o = (o_gamma / np.sqrt(o_var + EPS)).astype(f32)

    # q also absorbs the attention scale
    Wq_eff = (Wq * inv_q[:, None]) * SCALE
    bq = ((q_beta - q_mean * inv_q) * SCALE).astype(f32)
    Wk_eff = Wk * inv_k[:, None]
    bk = (k_beta - k_mean * inv_k).astype(f32)
    Wv_eff = Wv * inv_v[:, None]
    bv = (v_beta - v_mean * inv_v).astype(f32)
    Wo_eff = Wo * inv_o[:, None]
    bo_eff = ((bo - o_mean) * inv_o + o_beta).astype(f32)

    # lhsT layouts, pre-chunked to the exact SBUF tile shapes
    def chunk_T(w, kchunks):  # [O, C] -> [128, kchunks, O]  (WT[c, o] tiled)
        wT = np.ascontiguousarray(w.T.astype(f32))  # [C, O]
        c, o = wT.shape
        assert c == kchunks * 128
        return np.ascontiguousarray(
            wT.reshape(kchunks, 128, o).transpose(1, 0, 2)).astype(BF16)

    wqs = chunk_T(Wq_eff, 2)            # [128, 2, 256]
    wks = chunk_T(Wk_eff, 2)            # [128, 2, 256]
    wvs = chunk_T(Wv_eff, 2)            # [128, 2, 512]
    wos = chunk_T(Wo_eff, 4)            # [128, 4, 256]

    def chunk_bias(b, chunks):  # [C] -> [128, chunks]
        return np.ascontiguousarray(
            b.reshape(chunks, 128).T).astype(f32)

    bos = chunk_bias(bo_eff, 2)         # [128, 2]
    # q/k biases enter the projection as a K=1 ones-row matmul
    bqrow = bq.reshape(1, IDK).astype(BF16)  # [1, 256]
    bkrow = bk.reshape(1, IDK).astype(BF16)  # [1, 256]
    bvrow = bv.reshape(1, IDV).astype(BF16)  # [1, 512]  (h-major: c = h*64+dv)

    # exp-window tiles: indexed so that for j = 128*jc + p, attn[p, i] needs
    # expE[1023 + i - j] = win[p, h, (896 - 128*jc) + i]
    E = (np.asarray(pos_emb, dtype=np.float64) / SCALE)  # [N, HEADS]
    d = np.abs(np.arange(2047) - 1023)
    expE = np.exp(E[d, :])  # [2047, HEADS] float64
    idx = np.arange(WIN)[None, :] - np.arange(128)[:, None] + 127  # [128, WIN]
    win = expE[idx, :].transpose(0, 2, 1)  # [128, HEADS, WIN]
    win = np.ascontiguousarray(win).astype(BF16)

    shared = dict(wqs=wqs, wks=wks, wvs=wvs, wos=wos,
                  bqrow=bqrow, bkrow=bkrow, bos=bos, bvrow=bvrow, win=win)
    return shared


def _x_shard(x, i):
    # device consumes x as bf16 [128, 2, n] (channel chunks c = a*128 + p)
    xi = np.asarray(x[i], dtype=np.float32).reshape(2, 128, N).transpose(1, 0, 2)
    return np.ascontiguousarray(xi).astype(BF16)


def _build_nc():
    import concourse.bass as bass
    import concourse.mybir as mybir
    import concourse.tile as tile
    from concourse import bacc

    f32 = mybir.dt.float32
    bf16 = mybir.dt.bfloat16

    nc = bacc.Bacc(None, target_bir_lowering=False)

    x_ext = nc.declare_dram_parameter("x", [128, 2, N], bf16, isOutput=False)
    wqs_ext = nc.declare_dram_parameter("wqs", [128, 2, IDK], bf16, isOutput=False)
    wks_ext = nc.declare_dram_parameter("wks", [128, 2, IDK], bf16, isOutput=False)
    wvs_ext = nc.declare_dram_parameter("wvs", [128, 2, IDV], bf16, isOutput=False)
    wos_ext = nc.declare_dram_parameter("wos", [128, 4, DIM_OUT], bf16, isOutput=False)
    bqrow_ext = nc.declare_dram_parameter("bqrow", [1, IDK], bf16, isOutput=False)
    bkrow_ext = nc.declare_dram_parameter("bkrow", [1, IDK], bf16, isOutput=False)
    bos_ext = nc.declare_dram_parameter("bos", [128, 2], f32, isOutput=False)
    bvrow_ext = nc.declare_dram_parameter("bvrow", [1, IDV], bf16, isOutput=False)
    win_ext = nc.declare_dram_parameter("win", [128, HEADS, WIN], bf16, isOutput=False)
    out_ext = nc.declare_dram_parameter("out", [DIM_OUT, N], f32, isOutput=True)

    Exp = mybir.ActivationFunctionType.Exp
    Gelu = mybir.ActivationFunctionType.Gelu

    with tile.TileContext(nc) as tc:
        with (
            tc.tile_pool(name="consts", bufs=1) as consts,
            tc.tile_pool(name="scratch", bufs=2) as scratch,
            tc.tile_pool(name="attnp", bufs=6) as attnp,
            tc.tile_pool(name="normp", bufs=6) as normp,
            tc.tile_pool(name="psum", bufs=2, space="PSUM") as psum,
        ):
            # ---- load constants, in consumption order on one queue.  The
            # exp-window table (3.9MB, half the input bytes) is split into
            # per-pair chunks issued behind everything a chunk's first reader
            # doesn't need, so the early pipeline is never DMA-starved. ----
            xb = consts.tile([128, 2, N], bf16)
            nc.sync.dma_start(out=xb, in_=x_ext[:])
            wq = consts.tile([128, 2, IDK], bf16)
            nc.sync.dma_start(out=wq, in_=wqs_ext[:])
            wk = consts.tile([128, 2, IDK], bf16)
            nc.sync.dma_start(out=wk, in_=wks_ext[:])
            bqr = consts.tile([1, IDK], bf16)
            nc.sync.dma_start(out=bqr, in_=bqrow_ext[:])
            bkr = consts.tile([1, IDK], bf16)
            nc.sync.dma_start(out=bkr, in_=bkrow_ext[:])
            wv = consts.tile([128, 2, IDV], bf16)
            nc.sync.dma_start(out=wv, in_=wvs_ext[:])
            bvr = consts.tile([1, IDV], bf16)
            nc.sync.dma_start(out=bvr, in_=bvrow_ext[:])
            win = consts.tile([128, HEADS, WIN], bf16)
            win_ranges = [(768, 1920), (512, 768), (256, 512), (0, 256)]
            nc.sync.dma_start(out=win[:, 0:2, 768:1920],
                              in_=win_ext[:, 0:2, 768:1920])
            wo = consts.tile([128, 4, DIM_OUT], bf16)
            nc.sync.dma_start(out=wo, in_=wos_ext[:])
            bos = consts.tile([128, 2], f32)
            nc.sync.dma_start(out=bos, in_=bos_ext[:])
            for p in range(4):
                for (u0, u1) in win_ranges[1:] if p == 0 else win_ranges:
                    nc.sync.dma_start(out=win[:, 2 * p:2 * p + 2, u0:u1],
                                      in_=win_ext[:, 2 * p:2 * p + 2, u0:u1])
            ones1 = consts.tile([1, 512], bf16)
            nc.vector.memset(ones1, 1.0)
            # dummy exp: walrus inserts the exp table load before ACT's first
            # Exp -- placing one here pulls the ~2.7us load into the DMA wait
            # instead of the first attention pair's critical path
            warm = scratch.tile([1, 8], f32, tag="warm")
            nc.vector.memset(warm, 1.0)
            nc.scalar.activation(warm, warm, Exp)

            # ---- persistent intermediates ----
            q_sb = consts.tile([128, 2, N], bf16)   # [ (h,d) chunks, i ]
            k_sb = consts.tile([128, 2, N], bf16)   # [ (h,d) chunks, j ]
            # v columns 64:128 per head, columns 0:32 all-ones (sums come out
            # 32-replicated at base partition 0 -- safe for the custom-DVE
            # reciprocal), columns 32:64 zero (dark PE cells), out_u on
            # partitions 1:65.  M=65 keeps half the PE array dark vs M=128 --
            # same cycles, less power for the activity throttler to punish.
            v_aug = consts.tile([128, 8, HEADS, 128], bf16)  # [j-part, jc, h, one|0|dv]
            g_sb = consts.tile([128, 4, N], bf16)   # gelu input/output [(h,dv) chunks, i]

            nc.gpsimd.memset(v_aug[:, :, :, 0:32], 1.0)
            nc.gpsimd.memset(v_aug[:, :, :, 32:DV], 0.0)

            # ---- projections.  Each is one dots-ring pseudo-unit: 3 PE
            # matmuls into [:, 0:512] of a ring slot, evacuated by DVE so the
            # ACT queue stays exp-only mid-stream. ----
            def emit_qkproj(mc, w_t, b_r, dst, ic):
                ps_t = psum.tile([128, N], f32, tag="dots", bufs=2,
                                 name=f"qk_{mc}_{ic}")
                ps = ps_t[:, 0:512]
                for kc in range(2):
                    nc.tensor.matmul(
                        ps,
                        lhsT=w_t[:, kc, mc * 128:(mc + 1) * 128],
                        rhs=xb[:, kc, ic * 512:(ic + 1) * 512],
                        start=(kc == 0), stop=False)
                nc.tensor.matmul(
                    ps, lhsT=b_r[:, mc * 128:(mc + 1) * 128],
                    rhs=ones1[:, 0:512], start=False, stop=True)
                nc.vector.tensor_copy(dst[:, mc, ic * 512:(ic + 1) * 512], ps)

            def emit_vproj(jc):
                ps_t = psum.tile([128, N], f32, tag="dots", bufs=2,
                                 name=f"vps_{jc}")
                ps = ps_t[:, 0:512]
                for kc in range(2):
                    nc.tensor.matmul(
                        ps,
                        lhsT=xb[:, kc, jc * 128:(jc + 1) * 128],
                        rhs=wv[:, kc, :],
                        start=(kc == 0), stop=False)
                # + bias via ones-row x bvrow (K=1)
                nc.tensor.matmul(ps, lhsT=ones1[:, 0:128], rhs=bvr,
                                 start=False, stop=True)
                nc.vector.tensor_copy(
                    v_aug[:, jc, :, DV:128],
                    ps.rearrange("p (h d) -> p h d", h=HEADS))

            # PE warm-up: dense K=128 matmuls on garbage SBUF bridge the
            # input-DMA wait so HAM sees activity before real work lands
            wps = psum.tile([128, N], f32, tag="dots", bufs=2, name="warmps")
            for _ in range(4):
                nc.tensor.matmul(wps[:, 0:512], lhsT=q_sb[:, 0, 0:128],
                                 rhs=q_sb[:, 0, 0:512], start=True, stop=True)

            # minimal pre-attention batch: just what dots(p0, ic0, jc0..3)
            # needs; everything else interleaves into the attention stream
            emit_qkproj(0, wq, bqr, q_sb, 0)
            emit_qkproj(0, wk, bkr, k_sb, 0)

            # remaining projection units, interleaved into the attention
            # stream: (emit_fn, deadline = global unit index whose body needs
            # the result emitted first), sorted by deadline
            proj_queue = [
                (lambda: emit_qkproj(0, wk, bkr, k_sb, 1), 4),   # dots jc4
                (lambda: emit_vproj(0), 5),
                (lambda: emit_vproj(1), 5),
                (lambda: emit_vproj(2), 7),
                (lambda: emit_vproj(3), 7),
                (lambda: emit_qkproj(0, wq, bqr, q_sb, 1), 8),   # p0 ic1
                (lambda: emit_vproj(4), 9),
                (lambda: emit_vproj(5), 9),
                (lambda: emit_vproj(6), 11),
                (lambda: emit_vproj(7), 11),
                (lambda: emit_qkproj(1, wq, bqr, q_sb, 0), 31),  # p2+
                (lambda: emit_qkproj(1, wk, bkr, k_sb, 0), 31),
                (lambda: emit_qkproj(1, wk, bkr, k_sb, 1), 33),
                (lambda: emit_qkproj(1, wq, bqr, q_sb, 1), 39),
            ]

            # ---- attention: phases (pair, ihalf); units are jc 0..7 inside a
            # phase.  Two heads' K=32 dots matmuls share the PE array via
            # distinct 32-row groups (concurrent).  attn tiles pair two
            # consecutive jc so the window multiply runs at [128, 2048]
            # granularity; pairs alternate between DVE and GpSimd. ----
            import concourse.bass as bass_mod

            pending_norms = []

            def emit_norm_chain(p, ic, hl, ops):
                # g[dv, i-half] = out_u[dv, i-half] / sums[i-half]
                h = 2 * p + hl
                bc = normp.tile([DV, 512], f32, tag="bc", name=f"bc_{h}_{ic}")
                nc.vector.reciprocal_approx_fast(bc[0:32, :], ops[0:32, hl, :])
                nc.sync.dma_start(out=bc[32:DV, :], in_=bc[0:32, :])
                nc.vector.tensor_mul(
                    g_sb[hl * DV:hl * DV + DV, p, ic * 512:(ic + 1) * 512],
                    ops[DV:128, hl, :], bc)

            attnv_q = []

            def emit_attnv(p, ic, jc, acc, at, sl):
                for hl in range(2):
                    nc.tensor.matmul(
                        acc[:, hl, :],
                        lhsT=v_aug[:, jc, 2 * p + hl, :],
                        rhs=at[:, sl, hl, :],
                        start=(jc == 0), stop=(jc == 7))

            phases = [(p, ic) for p in range(4) for ic in range(2)]
            units = [(p, ic, jc) for (p, ic) in phases for jc in range(8)]
            acc_of = {}
            attn2 = None
            last_attn = None
            u = 0
            for p, ic, jc in units:
                h0, h1 = 2 * p, 2 * p + 1
                koff0, kch0 = (h0 % 4) * 32, h0 // 4
                koff1, kch1 = (h1 % 4) * 32, h1 // 4
                emitted = False
                while proj_queue and proj_queue[0][1] <= u:
                    proj_queue.pop(0)[0]()
                    emitted = True
                if proj_queue and not emitted and u % 3 == 2:
                    proj_queue.pop(0)[0]()
                if jc == 0:
                    acc_of[(p, ic)] = psum.tile(
                        [128, 2, 512], f32, tag="acc", bufs=2,
                        name=f"acc_{p}_{ic}")
                acc = acc_of[(p, ic)]
                if jc % 2 == 0:
                    attn2 = attnp.tile([128, 2, 2, 512], bf16, tag="attn",
                                       name=f"attn_{p}_{ic}_{jc}")
                # slot assignment: odd jc -> slot 0, even jc -> slot 1, so the
                # window view's pair dim can step +128 (odd jc has the
                # smaller window offset; negative AP steps avoided)
                sl = 1 - (jc % 2)
                dps = psum.tile([128, N], f32, tag="dots", bufs=2,
                                name=f"dots_{p}_{ic}_{jc}")
                nc.tensor.matmul(
                    dps[:, 0:512],
                    lhsT=k_sb[koff0:koff0 + 32, kch0, jc * 128:(jc + 1) * 128],
                    rhs=q_sb[koff0:koff0 + 32, kch0, ic * 512:(ic + 1) * 512],
                    start=True, stop=True, tile_position=(koff0, 0))
                nc.tensor.matmul(
                    dps[:, 512:1024],
                    lhsT=k_sb[koff1:koff1 + 32, kch1, jc * 128:(jc + 1) * 128],
                    rhs=q_sb[koff1:koff1 + 32, kch1, ic * 512:(ic + 1) * 512],
                    start=True, stop=True, tile_position=(koff1, 0))
                nc.scalar.activation(attn2[:, sl], dps.rearrange(
                    "p (h i) -> p h i", h=2), Exp)
                if jc % 2 == 1:
                    # one window multiply per unit-pair; attn@v consumers lag
                    # a pair behind, so waiting on both exps costs no latency
                    off = 896 - 128 * jc + 512 * ic
                    wv_view = win[:, h0, off:off + 512]
                    wv_quad = bass_mod.AP(
                        tensor=wv_view.tensor, offset=wv_view.offset,
                        ap=[list(wv_view.ap[0]), [128, 2], [WIN, 2], [1, 512]])
                    # wv_quad dim 1: index 0 = this (odd) jc's offset,
                    # index 1 = +128 = the even jc before it -- matching the
                    # slot convention above
                    eng = nc.vector if ((u // 2) % 2 == 0) else nc.gpsimd
                    eng.tensor_mul(attn2, attn2, wv_quad)
                    last_attn = attn2
                    attnv_q.append((p, ic, jc - 1, acc, attn2, 1))
                    attnv_q.append((p, ic, jc, acc, attn2, 0))
                    # attn@v lags TWO unit-pairs so even a GpSimd window
                    # multiply (~4us) is done before its attn@v reaches the
                    # head of the PE's in-order queue
                    while len(attnv_q) > 4:
                        emit_attnv(*attnv_q.pop(0))
                if pending_norms and jc >= 4:
                    emit_norm_chain(*pending_norms.pop(0))
                if jc == 7:
                    pending_norms += [(p, ic, 0, acc), (p, ic, 1, acc)]
                u += 1
            while attnv_q:
                emit_attnv(*attnv_q.pop(0))
            for fn, _ in proj_queue:
                fn()

            # ---- tail: the last phase's norms (DVE) run while ACT loads the
            # gelu table (warm-gelu gated on the last attn tile so the load
            # can't hoist into the exp stream), then gelu + out-projection
            # pipeline through the freed acc ring. ----
            out_r = out_ext[:].rearrange("(a p) n -> p a n", p=128)
            for args in pending_norms:
                emit_norm_chain(*args)
            pending_norms = []
            # zero-valued gelu bias derived from the LAST attn tile: a pure
            # scheduling gate keeping the gelus (and their table load) after
            # the final exp so the exp/gelu table sets swap exactly once.
            zg = scratch.tile([128, 1], f32, tag="zg")
            nc.vector.tensor_scalar_mul(zg, last_attn[:, 0, 0, 0:1], 0.0)
            fin = {mc: psum.tile([128, N], f32, tag="acc", bufs=2,
                                 name=f"fin_{mc}") for mc in range(2)}
            for cp in range(2):  # gelu chunk-pairs: heads 0-3, then 4-7
                gsl = g_sb[:, 2 * cp:2 * cp + 2, :]
                nc.scalar.activation(gsl, gsl, Gelu, bias=zg)
                for mc in range(2):
                    for ic in range(2):
                        for kc in (2 * cp, 2 * cp + 1):
                            nc.tensor.matmul(
                                fin[mc][:, ic * 512:(ic + 1) * 512],
                                lhsT=wo[:, kc, mc * 128:(mc + 1) * 128],
                                rhs=g_sb[:, kc, ic * 512:(ic + 1) * 512],
                                start=(kc == 0), stop=(kc == 3))
            for mc in range(2):
                o_sb = scratch.tile([128, N], f32, tag="osb",
                                    bufs=2, name=f"osb_{mc}")
                nc.vector.tensor_scalar_add(o_sb, fin[mc], bos[:, mc:mc + 1])
                nc.sync.dma_start(out=out_r[:, mc, :], in_=o_sb)

    nc.finalize()
    return nc


_NC_CACHE = None


def kernel(**inputs) -> np.ndarray:
    global _NC_CACHE
    from concourse.bass_utils import run_bass_kernel_spmd

    x = np.asarray(inputs["x"], dtype=np.float32)
    shared = _prep_host(**inputs)

    if _NC_CACHE is None:
        _NC_CACHE = _build_nc()
    nc = _NC_CACHE

    in_maps = [dict(x=_x_shard(x, i), **shared) for i in range(B)]
    res = run_bass_kernel_spmd(nc, in_maps, core_ids=list(range(B)))
    out = np.stack([res.results[i]["out"] for i in range(B)], axis=0)
    return out.astype(np.float32)


# revision 9
# speedup vs baseline: 1.4906x; 1.4136x over previous
"""Multi-head attention (BN-folded QKV + rel-pos bias + GELU + out-proj) on 8 TRN2 cores.

Data-parallel over batch (b=8 -> 1 batch element per core, no collectives).

All BatchNorms are eval-mode affine transforms folded into the projection
weights/biases on the host.  The additive Toeplitz position bias is folded in
multiplicatively after exp:  exp(dots + E) = exp(dots) * exp(E), with exp(E)
shipped as per-partition pre-shifted window tiles so each [j, i] tile of the
attention matrix multiplies a plain strided SBUF view.

Layout choice: dots are computed transposed (dotsT[j, i]) so q/k feed the
TensorEngine exactly as the projections produce them, v is produced already
transposed ([j, dv]) by swapping matmul operands, and a ones-column in v
makes the softmax denominators fall out of the attn@v matmul for free.
"""

import numpy as np
import ml_dtypes

HEADS, DK, DV = 8, 32, 64
DIM, N, DIM_OUT = 256, 1024, 256
IDK, IDV = DK * HEADS, DV * HEADS  # 256, 512
SCALE = DK ** -0.5
EPS = 1e-5
B = 8
WIN = 1920  # window tile width: covers i - 128*jc in [-896, 1023]

BF16 = ml_dtypes.bfloat16


def _prep_host(x, Wq, Wk, Wv, Wo, bo, pos_emb,
               q_gamma, q_beta, q_mean, q_var,
               k_gamma, k_beta, k_mean, k_var,
               v_gamma, v_beta, v_mean, v_var,
               o_gamma, o_beta, o_mean, o_var):
    f32 = np.float32
    inv_q = (q_gamma / np.sqrt(q_var + EPS)).astype(f32)
    inv_k = (k_gamma / np.sqrt(k_var + EPS)).astype(f32)
    inv_v = (v_gamma / np.sqrt(v_var + EPS)).astype(f32)
    inv_o = (o_gamma / np.sqrt(o_var + EPS)).astype(f32)

    # q also absorbs the attention scale
    Wq_eff = (Wq * inv_q[:, None]) * SCALE
    bq = ((q_beta - q_mean * inv_q) * SCALE).astype(f32)
    Wk_eff = Wk * inv_k[:, None]
    bk = (k_beta - k_mean * inv_k).astype(f32)
    Wv_eff = Wv * inv_v[:, None]
    bv = (v_beta - v_mean * inv_v).astype(f32)
    Wo_eff = Wo * inv_o[:, None]
    bo_eff = ((bo - o_mean) * inv_o + o_beta).astype(f32)

    # lhsT layouts, pre-chunked to the exact SBUF tile shapes
    def chunk_T(w, kchunks):  # [O, C] -> [128, kchunks, O]  (WT[c, o] tiled)
        wT = np.ascontiguousarray(w.T.astype(f32))  # [C, O]
        c, o = wT.shape
        assert c == kchunks * 128
        return np.ascontiguousarray(
            wT.reshape(kchunks, 128, o).transpose(1, 0, 2)).astype(BF16)

    wqs = chunk_T(Wq_eff, 2)            # [128, 2, 256]
    wks = chunk_T(Wk_eff, 2)            # [128, 2, 256]
    wvs = chunk_T(Wv_eff, 2)            # [128, 2, 512]
    wos = chunk_T(Wo_eff, 4)            # [128, 4, 256]

    def chunk_bias(b, chunks):  # [C] -> [128, chunks]
        return np.ascontiguousarray(
            b.reshape(chunks, 128).T).astype(f32)

    bos = chunk_bias(bo_eff, 2)         # [128, 2]
    # q/k biases enter the projection as a K=1 ones-row matmul
    bqrow = bq.reshape(1, IDK).astype(BF16)  # [1, 256]
    bkrow = bk.reshape(1, IDK).astype(BF16)  # [1, 256]
    bvrow = bv.reshape(1, IDV).astype(BF16)  # [1, 512]  (h-major: c = h*64+dv)

    # exp-window tiles: win[p, h, u] = exp(pos_emb[|u - p + 127 - 1023 + 1023|]...)
    # indexed so that for j = 128*jc + p, attn[p, i] needs
    # expE[1023 + i - j] = win[p, h, (896 - 128*jc) + i]
    E = (np.asarray(pos_emb, dtype=np.float64) / SCALE)  # [N, HEADS]
    d = np.abs(np.arange(2047) - 1023)
    expE = np.exp(E[d, :])  # [2047, HEADS] float64
    idx = np.arange(WIN)[None, :] - np.arange(128)[:, None] + 127  # [128, WIN]
    win = expE[idx, :].transpose(0, 2, 1)  # [128, HEADS, WIN]
    win = np.ascontiguousarray(win).astype(BF16)

    shared = dict(wqs=wqs, wks=wks, wvs=wvs, wos=wos,
                  bqrow=bqrow, bkrow=bkrow, bos=bos, bvrow=bvrow, win=win)
    return shared


def _x_shard(x, i):
    # device consumes x as bf16 [128, 2, n] (channel chunks c = a*128 + p)
    xi = np.asarray(x[i], dtype=np.float32).reshape(2, 128, N).transpose(1, 0, 2)
    return np.ascontiguousarray(xi).astype(BF16)


def _build_nc():
    import concourse.bass as bass
    import concourse.mybir as mybir
    import concourse.tile as tile
    from concourse import bacc

    f32 = mybir.dt.float32
    bf16 = mybir.dt.bfloat16

    nc = bacc.Bacc(None, target_bir_lowering=False)

    x_ext = nc.declare_dram_parameter("x", [128, 2, N], bf16, isOutput=False)
    wqs_ext = nc.declare_dram_parameter("wqs", [128, 2, IDK], bf16, isOutput=False)
    wks_ext = nc.declare_dram_parameter("wks", [128, 2, IDK], bf16, isOutput=False)
    wvs_ext = nc.declare_dram_parameter("wvs", [128, 2, IDV], bf16, isOutput=False)
    wos_ext = nc.declare_dram_parameter("wos", [128, 4, DIM_OUT], bf16, isOutput=False)
    bqrow_ext = nc.declare_dram_parameter("bqrow", [1, IDK], bf16, isOutput=False)
    bkrow_ext = nc.declare_dram_parameter("bkrow", [1, IDK], bf16, isOutput=False)
    bos_ext = nc.declare_dram_parameter("bos", [128, 2], f32, isOutput=False)
    bvrow_ext = nc.declare_dram_parameter("bvrow", [1, IDV], bf16, isOutput=False)
    win_ext = nc.declare_dram_parameter("win", [128, HEADS, WIN], bf16, isOutput=False)
    out_ext = nc.declare_dram_parameter("out", [DIM_OUT, N], f32, isOutput=True)

    Exp = mybir.ActivationFunctionType.Exp
    Gelu = mybir.ActivationFunctionType.Gelu

    with tile.TileContext(nc) as tc:
        with (
            tc.tile_pool(name="consts", bufs=1) as consts,
            tc.tile_pool(name="scratch", bufs=2) as scratch,
            tc.tile_pool(name="attnp", bufs=12) as attnp,
            tc.tile_pool(name="normp", bufs=6) as normp,
            tc.tile_pool(name="psum", bufs=2, space="PSUM") as psum,
        ):
            # ---- load constants, in consumption order on one queue.  The
            # exp-window table (3.9MB, half the input bytes) is split into
            # per-pair chunks issued behind everything a chunk's first reader
            # doesn't need, so the early pipeline is never DMA-starved. ----
            xb = consts.tile([128, 2, N], bf16)
            nc.sync.dma_start(out=xb, in_=x_ext[:])
            wq = consts.tile([128, 2, IDK], bf16)
            nc.sync.dma_start(out=wq, in_=wqs_ext[:])
            wk = consts.tile([128, 2, IDK], bf16)
            nc.sync.dma_start(out=wk, in_=wks_ext[:])
            bqr = consts.tile([1, IDK], bf16)
            nc.sync.dma_start(out=bqr, in_=bqrow_ext[:])
            bkr = consts.tile([1, IDK], bf16)
            nc.sync.dma_start(out=bkr, in_=bkrow_ext[:])
            wv = consts.tile([128, 2, IDV], bf16)
            nc.sync.dma_start(out=wv, in_=wvs_ext[:])
            bvr = consts.tile([1, IDV], bf16)
            nc.sync.dma_start(out=bvr, in_=bvrow_ext[:])
            win = consts.tile([128, HEADS, WIN], bf16)
            win_ranges = [(768, 1920), (512, 768), (256, 512), (0, 256)]
            nc.sync.dma_start(out=win[:, 0:2, 768:1920],
                              in_=win_ext[:, 0:2, 768:1920])
            wo = consts.tile([128, 4, DIM_OUT], bf16)
            nc.sync.dma_start(out=wo, in_=wos_ext[:])
            bos = consts.tile([128, 2], f32)
            nc.sync.dma_start(out=bos, in_=bos_ext[:])
            for p in range(4):
                for (u0, u1) in win_ranges[1:] if p == 0 else win_ranges:
                    nc.sync.dma_start(out=win[:, 2 * p:2 * p + 2, u0:u1],
                                      in_=win_ext[:, 2 * p:2 * p + 2, u0:u1])
            ones1 = consts.tile([1, 512], bf16)
            nc.vector.memset(ones1, 1.0)
            # dummy exp: walrus inserts the exp table load before ACT's first
            # Exp -- placing one here pulls the ~2.7us load into the DMA wait
            # instead of the first attention pair's critical path
            warm = scratch.tile([1, 8], f32, tag="warm")
            nc.vector.memset(warm, 1.0)
            nc.scalar.activation(warm, warm, Exp)

            # ---- persistent intermediates ----
            q_sb = consts.tile([128, 2, N], bf16)   # [ (h,d) chunks, i ]
            k_sb = consts.tile([128, 2, N], bf16)   # [ (h,d) chunks, j ]
            # v columns 64:128 per head, columns 0:32 all-ones (sums come out
            # 32-replicated at base partition 0 -- safe for the custom-DVE
            # reciprocal), columns 32:64 zero (dark PE cells), out_u on
            # partitions 1:65.  M=65 keeps half the PE array dark vs M=128 --
            # same cycles, less power for the activity throttler to punish.
            v_aug = consts.tile([128, 8, HEADS, 128], bf16)  # [j-part, jc, h, one|0|dv]
            g_sb = consts.tile([128, 4, N], bf16)   # gelu input/output [(h,dv) chunks, i]

            # ---- q/k projections (mc=0 first: heads 0-3 gate the first
            # attention pair).  The channel bias rides a K=1 ones-row matmul
            # so the evacuation is a table-free Copy on ACT -- the Identity
            # activation's table-set load would thrash against Exp's. ----
            def emit_qkproj(mc, w_t, b_r, dst, ic):
                ps_t = psum.tile([128, N], f32, tag="acc", bufs=2,
                                 name=f"qk_{mc}_{ic}")
                ps = ps_t[:, 0:512]
                for kc in range(2):
                    nc.tensor.matmul(
                        ps,
                        lhsT=w_t[:, kc, mc * 128:(mc + 1) * 128],
                        rhs=xb[:, kc, ic * 512:(ic + 1) * 512],
                        start=(kc == 0), stop=False)
                nc.tensor.matmul(
                    ps, lhsT=b_r[:, mc * 128:(mc + 1) * 128],
                    rhs=ones1[:, 0:512], start=False, stop=True)
                nc.scalar.copy(dst[:, mc, ic * 512:(ic + 1) * 512], ps)

            # ---- v projection, produced transposed: v_aug[j, (h, dv)] ----
            nc.vector.memset(v_aug[:, :, :, 0:32], 1.0)
            nc.vector.memset(v_aug[:, :, :, 32:DV], 0.0)

            def emit_vproj(jc):
                ps_t = psum.tile([128, N], f32, tag="acc", bufs=2,
                                 name=f"vps_{jc}")
                ps = ps_t[:, 0:512]
                for kc in range(2):
                    nc.tensor.matmul(
                        ps,
                        lhsT=xb[:, kc, jc * 128:(jc + 1) * 128],
                        rhs=wv[:, kc, :],
                        start=(kc == 0), stop=False)
                # + bias via ones-row x bvrow (K=1)
                nc.tensor.matmul(ps, lhsT=ones1[:, 0:128], rhs=bvr,
                                 start=False, stop=True)
                nc.vector.tensor_copy(
                    v_aug[:, jc, :, DV:128],
                    ps.rearrange("p (h d) -> p h d", h=HEADS))

            # ---- attention: head pairs; the two heads' K=32 dots matmuls share
            # the PE array via distinct 32-row groups (concurrent), two dots
            # tiles -> one exp + one pair-strided window multiply each ----
            pending_norms = []

            def emit_norm_chain(h, ops):
                # g[dv, i] = out_u[dv, i] / sums[i]  (+bv folded into v)
                # one chain per head now covers both ic halves (the
                # accumulator is one contiguous [128, 1024] tile): reciprocal
                # lands in the broadcast tile's first 32 partitions; one
                # SB->SB DMA widens to 64
                bc = normp.tile([DV, N], f32, tag="bc", name=f"bc_{h}")
                nc.vector.reciprocal_approx_fast(bc[0:32, :], ops[0:32, :])
                nc.sync.dma_start(out=bc[32:DV, :], in_=bc[0:32, :])
                nc.vector.tensor_mul(
                    g_sb[(h % 2) * DV:(h % 2) * DV + DV, h // 2, :],
                    ops[DV:128, :], bc)

            # PE warm-up: dense K=128 matmuls on garbage SBUF bridge the
            # input-DMA wait so HAM grants the 2.4GHz clock before real work
            # lands (results are overwritten by the first dots unit)
            wps = psum.tile([128, N], f32, tag="dots", bufs=2, name="warmps")
            for _ in range(10):
                nc.tensor.matmul(wps[:, 0:512], lhsT=q_sb[:, 0, 0:128],
                                 rhs=q_sb[:, 0, 0:512], start=True, stop=True)
            for mc in range(2):
                for (w_t, b_r, dst) in ((wq, bqr, q_sb), (wk, bkr, k_sb)):
                    for ic in range(2):
                        emit_qkproj(mc, w_t, b_r, dst, ic)
            for jc in range(8):
                emit_vproj(jc)

            units = [(p, jc) for p in range(4) for jc in range(8)]
            pair_state = {}
            attnv_q = []

            def emit_attnv(p, jc, at):
                # accumulators are one contiguous [128, 1024] tile per head
                # (both ic halves) so the norm chain runs once per head; the
                # matmuls stay 512-wide (a matmul dst may not span PSUM
                # banks -- N=1024 fails the s3d3_mm_num_elements ISA check)
                st = pair_state[p]
                for hl in range(2):
                    for ic in range(2):
                        nc.tensor.matmul(
                            st[hl][:, ic * 512:(ic + 1) * 512],
                            lhsT=v_aug[:, jc, 2 * p + hl, :],
                            rhs=at[:, hl, ic, :],
                            start=(jc == 0), stop=(jc == 7))

            import concourse.bass as bass_mod
            for p, jc in units:
                h0, h1 = 2 * p, 2 * p + 1
                koff0, kch0 = (h0 % 4) * 32, h0 // 4
                koff1, kch1 = (h1 % 4) * 32, h1 // 4
                if jc == 0:
                    pair_state[p] = [psum.tile([128, N], f32, tag="acc",
                                               bufs=2, name=f"acc_{h}")
                                     for h in (h0, h1)]
                off = 896 - 128 * jc
                # attn layout: [128, (head-half, ic, 512)] -- head-major so
                # each head's attn@v moving operand is one contiguous 1024
                attn = attnp.tile([128, 2, 2, 512], bf16, tag="attn",
                                  name=f"attn_{p}_{jc}")
                for ic in range(2):
                    dps = psum.tile([128, N], f32, tag="dots", bufs=2,
                                    name=f"dots_{p}_{jc}_{ic}")
                    nc.tensor.matmul(
                        dps[:, 0:512],
                        lhsT=k_sb[koff0:koff0 + 32, kch0, jc * 128:(jc + 1) * 128],
                        rhs=q_sb[koff0:koff0 + 32, kch0, ic * 512:(ic + 1) * 512],
                        start=True, stop=True, tile_position=(koff0, 0))
                    nc.tensor.matmul(
                        dps[:, 512:1024],
                        lhsT=k_sb[koff1:koff1 + 32, kch1, jc * 128:(jc + 1) * 128],
                        rhs=q_sb[koff1:koff1 + 32, kch1, ic * 512:(ic + 1) * 512],
                        start=True, stop=True, tile_position=(koff1, 0))
                    nc.scalar.activation(attn[:, :, ic, :], dps.rearrange(
                        "p (h i) -> p h i", h=2), Exp)
                # one window multiply for the whole (pair, jc) tile: the
                # attn@v consumers lag 2 units, so waiting on both exps here
                # costs no latency, and halving the DVE op count saves the
                # per-instruction overhead
                wv_view = win[:, h0, off:off + 512]
                wv_quad = bass_mod.AP(
                    tensor=wv_view.tensor, offset=wv_view.offset,
                    ap=[list(wv_view.ap[0]), [WIN, 2], [512, 2], [1, 512]])
                nc.vector.tensor_mul(attn, attn, wv_quad)
                last_attn = attn
                attnv_q.append((p, jc, attn))
                # attn@v two units behind: PE always has fresh dots work queued
                if len(attnv_q) > 2:
                    emit_attnv(*attnv_q.pop(0))
                # norms only after the lagging attn@v units of their pair have
                # been emitted (attnv lags 2 units -> safe from jc >= 2)
                if pending_norms and jc >= 2:
                    emit_norm_chain(*pending_norms.pop(0))
                if jc == 7:
                    pending_norms = [(h0, pair_state[p][0]),
                                     (h1, pair_state[p][1])]
            while attnv_q:
                emit_attnv(*attnv_q.pop(0))

            # ---- tail: pair 3's norms (DVE) overlap gelu + the kc 0-2 final
            # projection accumulation (heads 0-5 are normalized long ago; all
            # exps are done so the gelu table loads exactly once) ----
            out_r = out_ext[:].rearrange("(a p) n -> p a n", p=128)
            for args in pending_norms:
                emit_norm_chain(*args)
            pending_norms = []
            # zero-valued gelu bias derived from the LAST attn tile: a pure
            # scheduling gate.  Without it the framework hoists the g03 gelus
            # to their dep-ready point mid-exp-stream, thrashing the ACT
            # gelu/exp table sets twice; with it they land right after the
            # final exp, ahead of pair-3's norms, pulling the out-projection
            # forward.
            zg = scratch.tile([128, 1], f32, tag="zg")
            nc.vector.tensor_scalar_mul(zg, last_attn[:, 0, 0, 0:1], 0.0)
            fps_t = {}
            for ic in range(2):
                g03 = g_sb[:, 0:3, ic * 512:(ic + 1) * 512]
                nc.scalar.activation(g03, g03, Gelu, bias=zg)
                # the dots ring is free the instant the last exp reads it, so
                # the out-projection starts right after the g03 gelus instead
                # of waiting for pair-3's norms to release the acc ring
                fin = psum.tile([128, N], f32, tag="dots", bufs=2,
                                name=f"fin_{ic}")
                for mc in range(2):
                    fps = fin[:, mc * 512:(mc + 1) * 512]
                    fps_t[(mc, ic)] = fps
                    for kc in range(3):
                        nc.tensor.matmul(
                            fps,
                            lhsT=wo[:, kc, mc * 128:(mc + 1) * 128],
                            rhs=g_sb[:, kc, ic * 512:(ic + 1) * 512],
                            start=(kc == 0), stop=False)
            for ic in range(2):
                g3 = g_sb[:, 3:4, ic * 512:(ic + 1) * 512]
                nc.scalar.activation(g3, g3, Gelu)
                for mc in range(2):
                    fps = fps_t[(mc, ic)]
                    nc.tensor.matmul(
                        fps,
                        lhsT=wo[:, 3, mc * 128:(mc + 1) * 128],
                        rhs=g_sb[:, 3, ic * 512:(ic + 1) * 512],
                        start=False, stop=True)
                    o_sb = scratch.tile([128, 512], f32, tag="osb",
                                        bufs=4, name=f"osb_{mc}_{ic}")
                    nc.vector.tensor_scalar_add(o_sb, fps, bos[:, mc:mc + 1])
                    nc.sync.dma_start(out=out_r[:, mc, ic * 512:(ic + 1) * 512],
                                      in_=o_sb)

    nc.finalize()
    return nc


_NC_CACHE = None


def kernel(**inputs) -> np.ndarray:
    global _NC_CACHE
    from concourse.bass_utils import run_bass_kernel_spmd

    x = np.asarray(inputs["x"], dtype=np.float32)
    shared = _prep_host(**inputs)

    if _NC_CACHE is None:
        _NC_CACHE = _build_nc()
    nc = _NC_CACHE

    in_maps = [dict(x=_x_shard(x, i), **shared) for i in range(B)]
    res = run_bass_kernel_spmd(nc, in_maps, core_ids=list(range(B)))
    out = np.stack([res.results[i]["out"] for i in range(B)], axis=0)
    return out.astype(np.float32)

